# revision 7
# baseline (speedup 1.0000x reference)
"""Trainium2 Bass kernel for nn_Attn_Module (B=8, C=512, L=2048, CP=64).

Data-parallel over batch: each of the 8 NeuronCores computes one batch element's
full attention. No collectives.

Per-core math (b = batch element):
  v  = value_w @ x[b]                  [64, 2048]
  u  = Mq^T v,  Mq = (Qw/32)^T Kw      [64, 2048]  (E/32 = u^T v; k-pass eliminated)
  row bound b_l = -max_m (u^T v)[l, m] (stats pass, DVE reduces)
  E^T_biased[j, l] = [v; 1]^T [u; b]   (bias rides the matmul as a 65th K-row)
  P^T = exp(32 * E^T_biased)           bf16, directly in AV-ready [j, l] layout
  O65 = vT65^T @ P^T accumulated over j-tiles; vT65 = [gamma*v^T | ones-col]
        rows 0-63 = gamma*out_unnorm, row 64 = S2 (softmax denominator)
  out[0:64]  = O65[0:64] / S2 ;  out[64:128] = v
"""
import sys
import types

sys.path.insert(0, '/opt/trn_rl_repo')
sys.path.insert(0, '/root/.axon_site')

import numpy as np


def _install_ntff_hook():
    try:
        import antenv
    except ImportError:
        return
    if 'antenv.axon_hooks' in sys.modules:
        return
    mod = types.ModuleType('antenv.axon_hooks')
    mod._hook = None
    mod.set_axon_ntff_profile_hook = lambda h: setattr(mod, '_hook', h)
    mod.get_axon_ntff_profile_hook = lambda: mod._hook
    sys.modules['antenv.axon_hooks'] = mod
    antenv.axon_hooks = mod
    try:
        from trn_agent_boot.trn_boot import _ntff_profile_via_ctypes
        mod.set_axon_ntff_profile_hook(_ntff_profile_via_ctypes('/opt/axon/libaxon_pjrt.so'))
    except Exception:
        pass


_install_ntff_hook()

import concourse.bacc as bacc
import concourse.mybir as mybir
from concourse.bass_utils import run_bass_kernel_spmd
from concourse.tile import TileContext

F32 = mybir.dt.float32
F32R = mybir.dt.float32r
BF16 = mybir.dt.bfloat16

B, C, L, CP = 8, 512, 2048, 64
NLT = L // 128     # 16 l-tiles
NJT = L // 128     # 16 j-tiles
NLC = L // 512     # 4 chunks
NG = NJT // 2      # 8 j-groups of 2 tiles (one [128,1024] PSUM pair each)
SCALE = 32.0
N_WARMUP = 48


def f32r_round(a):
    """Round fp32 array to the float32r grid (RNE on low 12 mantissa bits, sign-magnitude)."""
    a = np.ascontiguousarray(a, np.float32)
    xi = a.view(np.int32)
    sign = xi & np.int32(-2**31)
    mag = (xi & np.int32(0x7FFFFFFF)).astype(np.int64)
    add = 1 << 11
    mr = mag + add
    ties = (mag & ((1 << 12) - 1)) == add
    mr = np.where(ties & (((mag >> 12) & 1) == 0), mag, mr)
    mr &= ~((1 << 12) - 1)
    return (sign | mr.astype(np.int32)).view(np.float32).reshape(a.shape)


def build_nc(gamma: float, debug: bool = False):
    nc = bacc.Bacc()
    x_p = nc.declare_dram_parameter('x', [C, L], F32R, isOutput=False)
    vwT_p = nc.declare_dram_parameter('vwT', [C, CP], F32R, isOutput=False)
    mq_p = nc.declare_dram_parameter('mq', [CP, CP], F32R, isOutput=False)
    id_p = nc.declare_dram_parameter('ident', [128, 128], F32R, isOutput=False)
    out_p = nc.declare_dram_parameter('out', [128, L], F32, isOutput=True)
    if debug:
        dbg_q_p = nc.declare_dram_parameter('dbg_q', [65, L], F32, isOutput=True)
        dbg_k_p = nc.declare_dram_parameter('dbg_k', [65, L], F32, isOutput=True)
        dbg_sh_p = nc.declare_dram_parameter('dbg_sh', [128, 3 * NLT], F32, isOutput=True)
        dbg_ns_p = nc.declare_dram_parameter('dbg_ns', [128, 32], F32, isOutput=True)

    with TileContext(nc) as tc:
        with tc.tile_pool(name='sb', bufs=1) as sb, \
             tc.tile_pool(name='pt', bufs=4) as ptp, \
             tc.tile_pool(name='nr', bufs=4) as nrp, \
             tc.tile_pool(name='et', bufs=2, space='PSUM') as etp, \
             tc.tile_pool(name='st', bufs=1, space='PSUM') as stp, \
             tc.tile_pool(name='oo', bufs=2, space='PSUM') as oop:

            # ---------- small loads first (weights before x) ----------
            ident = sb.tile([128, 128], F32R, tag='ident')
            nc.sync.dma_start(ident[:], id_p[:])
            vw = sb.tile([128, 4 * CP], F32R, tag='vw')
            for kt in range(4):
                nc.sync.dma_start(vw[:, kt * CP:(kt + 1) * CP], vwT_p[kt * 128:(kt + 1) * 128, :])
            mq = sb.tile([64, CP], F32R, tag='mq')
            nc.sync.dma_start(mq[:], mq_p[:])
            actwarm = sb.tile([1, 8], F32, tag='actwarm')
            nc.scalar.activation(actwarm[:], ident[0:1, 0:8].bitcast(F32),
                                 mybir.ActivationFunctionType.Exp, bias=0.0, scale=0.0)

            # ---------- PE warmup (keeps the HAM clock-gate open during x load) ----------
            for w in range(N_WARMUP):
                wps = oop.tile([64, 128], F32, tag='oo', name=f'warm{w}')
                nc.tensor.matmul(wps[:], ident[:, 0:64], ident[:], start=True, stop=True)

            # ---------- x load: 8 tiles [128, 1024], each as 2 partition-half DMAs ----------
            xc = [[sb.tile([128, 1024], F32R, tag=f'x{kt}_{lh}', name=f'x{kt}_{lh}')
                   for lh in range(2)] for kt in range(4)]
            for lh in range(2):
                for kt in range(4):
                    for ph in range(2):
                        nc.sync.dma_start(
                            xc[kt][lh][ph * 64:(ph + 1) * 64, :],
                            x_p[kt * 128 + ph * 64:kt * 128 + (ph + 1) * 64,
                                lh * 1024:(lh + 1) * 1024])

            # ---------- persistent SBUF ----------
            K65 = sb.tile([65, L], F32R, tag='K65')   # rows 0:64 = v, row 64 = ones
            Q65 = sb.tile([65, L], F32R, tag='Q65')   # rows 0:64 = u, row 64 = brow
            nc.gpsimd.memset(K65[64:65, :].bitcast(F32), 1.0)
            vt65 = sb.tile([128, NJT * 65], BF16, tag='vt65')
            ones_col = vt65[:].rearrange('p (a b) -> p a b', b=65)[:, :, 64:65]
            nc.gpsimd.memset(ones_col, 1.0)
            statsH = sb.tile([128, 3 * NLT], F32, tag='statsH')   # per l-tile: m01max, m2max, m3max
            negst = sb.tile([128, 32], F32, tag='negst')          # col lt = -max (padded to 32)
            statsT = sb.tile([128, 32], F32, tag='statsT')        # 32x32 block transpose of negst

            # ---------- v / u per chunk ----------
            def emit_v(lc):
                pv = oop.tile([64, 512], F32, tag='oo', name=f'pv{lc}')
                lh, c2 = lc // 2, lc % 2
                for kt in range(4):
                    nc.tensor.matmul(pv[:], vw[:, kt * CP:(kt + 1) * CP],
                                     xc[kt][lh][:, c2 * 512:(c2 + 1) * 512],
                                     start=(kt == 0), stop=(kt == 3))
                nc.scalar.copy(K65[0:64, lc * 512:(lc + 1) * 512], pv[:])

            def emit_u(lc):
                pu = oop.tile([64, 512], F32, tag='oo', name=f'pu{lc}')
                nc.tensor.matmul(pu[:], mq[:], K65[0:64, lc * 512:(lc + 1) * 512],
                                 start=True, stop=True)
                nc.scalar.copy(Q65[0:64, lc * 512:(lc + 1) * 512], pu[:])

            # ---------- vT65 (gamma * v^T | ones col), bf16; group g covers jt 8g..8g+7 ----------
            def emit_T(g):
                pvt = oop.tile([128, 512], F32R, tag='oo', name=f'pvt{g}')
                for bi in range(8):
                    jt = g * 8 + bi
                    nc.tensor.transpose(pvt[:, bi * 64:(bi + 1) * 64],
                                        K65[0:64, jt * 128:(jt + 1) * 128],
                                        ident[0:64, 0:64])
                dst = vt65[:, g * 8 * 65:].rearrange('p (a b) -> p a b', b=65)[:, 0:8, 0:64]
                nc.scalar.mul(dst, pvt[:].rearrange('p (a b) -> p a b', b=64), float(gamma))

            # ---------- stats units: per l-tile, h0 = max over m[0:1024], h1 = m[1024:2048] ----------
            def stats_h0(lt, pool, tag):
                sg = pool.tile([128, 1024], F32, tag=tag, name=f'sA{lt}')
                for i, mc in enumerate((0, 1)):
                    nc.tensor.matmul(sg[:, i * 512:(i + 1) * 512],
                                     Q65[0:64, lt * 128:(lt + 1) * 128],
                                     K65[0:64, mc * 512:(mc + 1) * 512],
                                     start=True, stop=True)
                nc.vector.reduce_max(statsH[:, 3 * lt:3 * lt + 1], sg[:],
                                     axis=mybir.AxisListType.X)

            def stats_h1(lt, pool, tag):
                sg = pool.tile([128, 1024], F32, tag=tag, name=f'sB{lt}')
                for i, mc in enumerate((2, 3)):
                    nc.tensor.matmul(sg[:, i * 512:(i + 1) * 512],
                                     Q65[0:64, lt * 128:(lt + 1) * 128],
                                     K65[0:64, mc * 512:(mc + 1) * 512],
                                     start=True, stop=True)
                    nc.vector.reduce_max(statsH[:, 3 * lt + 1 + i:3 * lt + 2 + i],
                                         sg[:, i * 512:(i + 1) * 512],
                                         axis=mybir.AxisListType.X)

            def emit_brow(lc):
                # combine the 3 partial maxes -> negst cols [4lc:4lc+4], negated
                src = statsH[:, 12 * lc:12 * lc + 12].rearrange('p (lt s) -> p lt s', s=3)
                nc.vector.reduce_max(negst[:, 4 * lc:4 * lc + 4], src,
                                     axis=mybir.AxisListType.X, negate=True)
                # 32x32 block transpose: negst[32a+j, lt] -> statsT[32a+lt, j]
                nc.vector.transpose(statsT[:], negst[:])
                # statsT[32a + lt, j] -> Q65 row 64, col lt*128 + 32a + j  (lt global)
                dst_all = Q65[64:65, lc * 512:(lc + 1) * 512].bitcast(F32).rearrange(
                    'p (i a j) -> p i a j', i=4, a=4)
                for a in range(4):
                    src_ap = statsT[32 * a + 4 * lc:32 * a + 4 * lc + 4, :].rearrange(
                        'q (z j) -> q z j', z=1)
                    nc.sync.dma_start(dst_all[:, :, a:a + 1, :], src_ap)

            # ---------- prologue ----------
            emit_v(0)
            emit_u(0)
            emit_v(1)
            emit_u(1)
            emit_T(0)
            for lt in range(4):
                stats_h0(lt, etp, 'et')
            emit_v(2)
            emit_u(2)
            emit_v(3)
            emit_u(3)
            emit_T(1)
            for lt in range(4):
                stats_h1(lt, etp, 'et')
            emit_brow(0)
            # v output channels (final): overlap store with the body
            nc.scalar.dma_start(out_p[64:128, :], K65[0:64, :].bitcast(F32))

            # ---------- body ----------
            o65 = [None] * NLC
            pts = [None] * NG

            def emit_et_group(lc, g):
                eg = etp.tile([128, 1024], F32, tag='et', name=f'e{lc}_{g}')
                for i in range(2):
                    jt = 2 * g + i
                    nc.tensor.matmul(eg[:, i * 512:(i + 1) * 512],
                                     K65[0:65, jt * 128:(jt + 1) * 128],
                                     Q65[0:65, lc * 512:(lc + 1) * 512],
                                     start=True, stop=True)
                pg = ptp.tile([128, 1024], BF16, tag='pt', name=f'p{lc}_{g}')
                nc.scalar.activation(pg[:], eg[:], mybir.ActivationFunctionType.Exp,
                                     bias=0.0, scale=SCALE)
                pts[g] = pg

            def emit_av(lc, g):
                for i in range(2):
                    jt = 2 * g + i
                    nc.tensor.matmul(o65[lc][:], vt65[:, jt * 65:(jt + 1) * 65],
                                     pts[g][:, i * 512:(i + 1) * 512],
                                     start=(jt == 0), stop=(jt == NJT - 1))

            def emit_norm(lc):
                s2s = nrp.tile([1, 512], F32, tag='nr', name=f's2_{lc}')
                nc.scalar.copy(s2s[:], o65[lc][64:65, :])
                r1 = nrp.tile([1, 512], F32, tag='nr', name=f'r1_{lc}')
                nc.vector.reciprocal_approx_fast(r1[:], s2s[:])
                r2 = nrp.tile([64, 512], F32, tag='nr', name=f'r2_{lc}')
                nc.gpsimd.partition_broadcast(r2[:], r1[:])
                ofin = nrp.tile([64, 512], F32, tag='nr', name=f'of{lc}')
                nc.vector.tensor_tensor(ofin[:], o65[lc][0:64, :], r2[:],
                                        op=mybir.AluOpType.mult)
                nc.sync.dma_start(out_p[0:64, lc * 512:(lc + 1) * 512], ofin[:])

            for lc in range(NLC):
                o65[lc] = oop.tile([65, 512], F32, tag='oo', name=f'o65_{lc}')
                # stats for lc+1 spread through this iteration (units: 4 lt x (h0, h1))
                nlc = lc + 1
                emit_et_group(lc, 0)
                for g in range(1, NG):
                    emit_et_group(lc, g)
                    if nlc < NLC:
                        lt = 4 * nlc + (g - 1) // 2
                        (stats_h0 if (g - 1) % 2 == 0 else stats_h1)(lt, stp, 'st')
                    emit_av(lc, g - 1)
                if nlc < NLC:
                    stats_h1(4 * nlc + 3, stp, 'st')
                emit_av(lc, NG - 1)
                if nlc < NLC:
                    emit_brow(nlc)
                emit_norm(lc)

            if debug:
                nc.sync.dma_start(dbg_q_p[:], Q65[:].bitcast(F32))
                nc.sync.dma_start(dbg_k_p[:], K65[:].bitcast(F32))
                nc.sync.dma_start(dbg_sh_p[:], statsH[:])
                nc.sync.dma_start(dbg_ns_p[:], negst[:])

    nc.finalize()
    return nc


_cache = {}


def _get_nc(gamma: float):
    key = float(gamma)
    if key not in _cache:
        _cache[key] = build_nc(key)
    return _cache[key]


def _in_maps(inputs):
    x = np.asarray(inputs['x'], np.float32)
    vwT = f32r_round(np.asarray(inputs['value_w'], np.float32).T)
    qw = np.asarray(inputs['query_w'], np.float32)
    kw = np.asarray(inputs['key_w'], np.float32)
    mq = f32r_round((qw.T / SCALE) @ kw)
    ident = np.eye(128, dtype=np.float32)
    xs = f32r_round(x[..., 0])
    return [
        {'x': np.ascontiguousarray(xs[b]), 'vwT': vwT, 'mq': mq, 'ident': ident}
        for b in range(B)
    ]


def kernel(x, value_w, value_b, query_w, query_b, key_w, key_b, gamma):
    gamma_f = float(np.asarray(gamma).reshape(-1)[0])
    nc = _get_nc(gamma_f)
    maps = _in_maps(dict(x=x, value_w=value_w, query_w=query_w, key_w=key_w))
    res = run_bass_kernel_spmd(nc, maps, core_ids=list(range(B)), trace=False)
    out = np.stack([res.results[b]['out'] for b in range(B)], axis=0)
    return out[..., None].astype(np.float32)


def run_traced(inputs):
    gamma_f = float(np.asarray(inputs['gamma']).reshape(-1)[0])
    nc = _get_nc(gamma_f)
    maps = _in_maps(inputs)
    res = run_bass_kernel_spmd(nc, maps, core_ids=list(range(B)), trace=True)
    out = np.stack([res.results[b]['out'] for b in range(B)], axis=0)
    return out[..., None].astype(np.float32), res.exec_time_ns


# revision 9
# speedup vs baseline: 1.1082x; 1.1082x over previous
"""Trainium2 Bass kernel for nn_Attn_Module (B=8, C=512, L=2048, CP=64).

Data-parallel over batch: each of the 8 NeuronCores computes one batch element's
full attention. No collectives.

Per-core math (b = batch element):
  v  = value_w @ x[b]                  [64, 2048]
  u  = Mq^T v,  Mq = (Qw/32)^T Kw      [64, 2048]  (E/32 = u^T v; k-pass eliminated)
  row bound b_l = -max_m (u^T v)[l, m] (stats pass, DVE reduces)
  E^T_biased[j, l] = [v; 1]^T [u; b]   (bias rides the matmul as a 65th K-row)
  P^T = exp(32 * E^T_biased)           bf16, directly in AV-ready [j, l] layout
  O65 = vT65^T @ P^T accumulated over j-tiles; vT65 = [gamma*v^T | ones-col]
        rows 0-63 = gamma*out_unnorm, row 64 = S2 (softmax denominator)
  out[0:64]  = O65[0:64] / S2 ;  out[64:128] = v
"""
import sys
import types

sys.path.insert(0, '/opt/trn_rl_repo')
sys.path.insert(0, '/root/.axon_site')

import numpy as np


def _install_ntff_hook():
    try:
        import antenv
    except ImportError:
        return
    if 'antenv.axon_hooks' in sys.modules:
        return
    mod = types.ModuleType('antenv.axon_hooks')
    mod._hook = None
    mod.set_axon_ntff_profile_hook = lambda h: setattr(mod, '_hook', h)
    mod.get_axon_ntff_profile_hook = lambda: mod._hook
    sys.modules['antenv.axon_hooks'] = mod
    antenv.axon_hooks = mod
    try:
        from trn_agent_boot.trn_boot import _ntff_profile_via_ctypes
        mod.set_axon_ntff_profile_hook(_ntff_profile_via_ctypes('/opt/axon/libaxon_pjrt.so'))
    except Exception:
        pass


_install_ntff_hook()

import concourse.bacc as bacc
import concourse.mybir as mybir
from concourse.bass_utils import run_bass_kernel_spmd
from concourse.tile import TileContext

F32 = mybir.dt.float32
F32R = mybir.dt.float32r
BF16 = mybir.dt.bfloat16

B, C, L, CP = 8, 512, 2048, 64
NLT = L // 128     # 16 l-tiles
NJT = L // 128     # 16 j-tiles
NLC = L // 512     # 4 chunks
NG = NJT // 2      # 8 j-groups of 2 tiles (one [128,1024] PSUM pair each)
SCALE = 32.0
N_WARMUP = 48


def f32r_round(a):
    """Round fp32 array to the float32r grid (RNE on low 12 mantissa bits, sign-magnitude)."""
    a = np.ascontiguousarray(a, np.float32)
    xi = a.view(np.int32)
    sign = xi & np.int32(-2**31)
    mag = (xi & np.int32(0x7FFFFFFF)).astype(np.int64)
    add = 1 << 11
    mr = mag + add
    ties = (mag & ((1 << 12) - 1)) == add
    mr = np.where(ties & (((mag >> 12) & 1) == 0), mag, mr)
    mr &= ~((1 << 12) - 1)
    return (sign | mr.astype(np.int32)).view(np.float32).reshape(a.shape)


def build_nc(gamma: float, debug: bool = False):
    nc = bacc.Bacc()
    x_p = nc.declare_dram_parameter('x', [C, L], F32R, isOutput=False)
    vwT_p = nc.declare_dram_parameter('vwT', [C, CP], F32R, isOutput=False)
    mq_p = nc.declare_dram_parameter('mq', [CP, CP], F32R, isOutput=False)
    id_p = nc.declare_dram_parameter('ident', [128, 128], F32R, isOutput=False)
    out_p = nc.declare_dram_parameter('out', [128, L], F32, isOutput=True)
    if debug:
        dbg_q_p = nc.declare_dram_parameter('dbg_q', [65, L], F32, isOutput=True)
        dbg_k_p = nc.declare_dram_parameter('dbg_k', [65, L], F32, isOutput=True)
        dbg_sh_p = nc.declare_dram_parameter('dbg_sh', [128, 3 * NLT], F32, isOutput=True)
        dbg_ns_p = nc.declare_dram_parameter('dbg_ns', [128, 32], F32, isOutput=True)

    with TileContext(nc) as tc:
        with tc.tile_pool(name='sb', bufs=1) as sb, \
             tc.tile_pool(name='pt', bufs=3) as ptp, \
             tc.tile_pool(name='so', bufs=2) as sop, \
             tc.tile_pool(name='nr', bufs=4) as nrp, \
             tc.tile_pool(name='wk', bufs=3, space='PSUM') as wkp, \
             tc.tile_pool(name='oo', bufs=2, space='PSUM') as oop:

            # ---------- small loads first (weights before x) ----------
            ident = sb.tile([128, 128], F32R, tag='ident')
            nc.sync.dma_start(ident[:], id_p[:])
            vw = sb.tile([128, 4 * CP], F32R, tag='vw')
            for kt in range(4):
                nc.sync.dma_start(vw[:, kt * CP:(kt + 1) * CP], vwT_p[kt * 128:(kt + 1) * 128, :])
            mq = sb.tile([64, CP], F32R, tag='mq')
            nc.sync.dma_start(mq[:], mq_p[:])
            actwarm = sb.tile([1, 8], F32, tag='actwarm')
            nc.scalar.activation(actwarm[:], ident[0:1, 0:8].bitcast(F32),
                                 mybir.ActivationFunctionType.Exp, bias=0.0, scale=0.0)

            # ---------- PE warmup (keeps the HAM clock-gate open during x load) ----------
            for w in range(N_WARMUP):
                wps = oop.tile([64, 128], F32, tag='oo', name=f'warm{w}')
                nc.tensor.matmul(wps[:], ident[:, 0:64], ident[:], start=True, stop=True)

            # ---------- x load: 8 tiles [128, 1024], each as 2 partition-half DMAs ----------
            xc = [[sb.tile([128, 1024], F32R, tag=f'x{kt}_{lh}', name=f'x{kt}_{lh}')
                   for lh in range(2)] for kt in range(4)]
            for lh in range(2):
                for kt in range(4):
                    for ph in range(2):
                        nc.sync.dma_start(
                            xc[kt][lh][ph * 64:(ph + 1) * 64, :],
                            x_p[kt * 128 + ph * 64:kt * 128 + (ph + 1) * 64,
                                lh * 1024:(lh + 1) * 1024])

            # ---------- persistent SBUF ----------
            K65 = sb.tile([65, L], F32R, tag='K65')   # rows 0:64 = v, row 64 = ones
            Q65 = sb.tile([65, L], F32R, tag='Q65')   # rows 0:64 = u, row 64 = brow
            nc.gpsimd.memset(K65[64:65, :].bitcast(F32), 1.0)
            vt65 = sb.tile([128, NJT * 65], BF16, tag='vt65')
            ones_col = vt65[:].rearrange('p (a b) -> p a b', b=65)[:, :, 64:65]
            nc.gpsimd.memset(ones_col, 1.0)
            statsH = sb.tile([128, 3 * NLT], F32, tag='statsH')   # per l-tile: up to 3 partial maxes
            nc.gpsimd.memset(statsH[:], -3.0e38)
            negst = sb.tile([128, 32], F32, tag='negst')          # col lt = -max (padded to 32)
            statsT = sb.tile([128, 32], F32, tag='statsT')        # 32x32 block transpose of negst

            # ---------- v / u per chunk ----------
            def emit_v(lc):
                pv = oop.tile([64, 512], F32, tag='oo', name=f'pv{lc}')
                lh, c2 = lc // 2, lc % 2
                for kt in range(4):
                    nc.tensor.matmul(pv[:], vw[:, kt * CP:(kt + 1) * CP],
                                     xc[kt][lh][:, c2 * 512:(c2 + 1) * 512],
                                     start=(kt == 0), stop=(kt == 3))
                nc.scalar.copy(K65[0:64, lc * 512:(lc + 1) * 512], pv[:])

            def emit_u(lc):
                pu = oop.tile([64, 512], F32, tag='oo', name=f'pu{lc}')
                nc.tensor.matmul(pu[:], mq[:], K65[0:64, lc * 512:(lc + 1) * 512],
                                 start=True, stop=True)
                nc.scalar.copy(Q65[0:64, lc * 512:(lc + 1) * 512], pu[:])

            # ---------- vT65 (gamma * v^T | ones col), bf16; group g covers jt 8g..8g+7 ----------
            def emit_T(g):
                pvt = oop.tile([128, 512], F32R, tag='oo', name=f'pvt{g}')
                for bi in range(8):
                    jt = g * 8 + bi
                    nc.tensor.transpose(pvt[:, bi * 64:(bi + 1) * 64],
                                        K65[0:64, jt * 128:(jt + 1) * 128],
                                        ident[0:64, 0:64])
                dst = vt65[:, g * 8 * 65:].rearrange('p (a b) -> p a b', b=65)[:, 0:8, 0:64]
                nc.scalar.mul(dst, pvt[:].rearrange('p (a b) -> p a b', b=64), float(gamma))

            # ---------- stats units: per l-tile, h0 = max over m[0:1024], h1 = m[1024:2048] ----------
            def stats_mm(lt, h):
                sg = wkp.tile([128, 1024], F32, tag='wk', name=f's{h}_{lt}')
                for i in range(2):
                    mc = 2 * h + i
                    nc.tensor.matmul(sg[:, i * 512:(i + 1) * 512],
                                     Q65[0:64, lt * 128:(lt + 1) * 128],
                                     K65[0:64, mc * 512:(mc + 1) * 512],
                                     start=True, stop=True)
                return sg

            def stats_red(lt, h, sg):
                # one [128,1024] DVE reduce -> statsH col 3lt+h
                nc.vector.reduce_max(statsH[:, 3 * lt + h:3 * lt + h + 1], sg[:],
                                     axis=mybir.AxisListType.X)

            def stats_red_split(lt, h, sg):
                # two single-bank reduces (lower latency; cols 3lt+1, 3lt+2)
                for i in range(2):
                    nc.vector.reduce_max(statsH[:, 3 * lt + 1 + i:3 * lt + 2 + i],
                                         sg[:, i * 512:(i + 1) * 512],
                                         axis=mybir.AxisListType.X)


            def emit_brow(lc):
                # combine the 3 partial maxes -> negst cols [4lc:4lc+4], negated
                src = statsH[:, 12 * lc:12 * lc + 12].rearrange('p (lt s) -> p lt s', s=3)
                nc.vector.reduce_max(negst[:, 4 * lc:4 * lc + 4], src,
                                     axis=mybir.AxisListType.X, negate=True)
                # 32x32 block transpose: negst[32a+j, lt] -> statsT[32a+lt, j]
                nc.vector.transpose(statsT[:], negst[:])
                # statsT[32a + lt, j] -> Q65 row 64, col lt*128 + 32a + j  (lt global)
                dst_all = Q65[64:65, lc * 512:(lc + 1) * 512].bitcast(F32).rearrange(
                    'p (i a j) -> p i a j', i=4, a=4)
                for a in range(4):
                    src_ap = statsT[32 * a + 4 * lc:32 * a + 4 * lc + 4, :].rearrange(
                        'q (z j) -> q z j', z=1)
                    nc.sync.dma_start(dst_all[:, :, a:a + 1, :], src_ap)

            # ---------- prologue ----------
            emit_v(0)
            emit_u(0)
            emit_v(1)
            emit_u(1)
            emit_T(0)
            for lt in range(4):
                sg = stats_mm(lt, 0)
                stats_red(lt, 0, sg)
            emit_v(2)
            emit_u(2)
            emit_v(3)
            emit_u(3)
            emit_T(1)
            for lt in range(4):
                sg = stats_mm(lt, 1)
                stats_red_split(lt, 1, sg)
            emit_brow(0)
            # v output channels (final): overlap store with the body
            nc.scalar.dma_start(out_p[64:128, :], K65[0:64, :].bitcast(F32))

            # ---------- body ----------
            o65 = [None] * NLC
            pts = [None] * NG

            def emit_et_group(lc, g):
                eg = wkp.tile([128, 1024], F32, tag='wk', name=f'e{lc}_{g}')
                for i in range(2):
                    jt = 2 * g + i
                    nc.tensor.matmul(eg[:, i * 512:(i + 1) * 512],
                                     K65[0:65, jt * 128:(jt + 1) * 128],
                                     Q65[0:65, lc * 512:(lc + 1) * 512],
                                     start=True, stop=True)
                pg = ptp.tile([128, 1024], BF16, tag='pt', name=f'p{lc}_{g}')
                nc.scalar.activation(pg[:], eg[:], mybir.ActivationFunctionType.Exp,
                                     bias=0.0, scale=SCALE)
                pts[g] = pg

            def emit_av(lc, g):
                for i in range(2):
                    jt = 2 * g + i
                    nc.tensor.matmul(o65[lc][:], vt65[:, jt * 65:(jt + 1) * 65],
                                     pts[g][:, i * 512:(i + 1) * 512],
                                     start=(jt == 0), stop=(jt == NJT - 1))

            def emit_norm(lc):
                s2s = nrp.tile([1, 512], F32, tag='nr', name=f's2_{lc}')
                nc.scalar.copy(s2s[:], o65[lc][64:65, :])
                r1 = nrp.tile([1, 512], F32, tag='nr', name=f'r1_{lc}')
                nc.vector.reciprocal_approx_fast(r1[:], s2s[:])
                r2 = nrp.tile([64, 512], F32, tag='nr', name=f'r2_{lc}')
                nc.gpsimd.partition_broadcast(r2[:], r1[:])
                ofin = nrp.tile([64, 512], F32, tag='nr', name=f'of{lc}')
                nc.vector.tensor_tensor(ofin[:], o65[lc][0:64, :], r2[:],
                                        op=mybir.AluOpType.mult)
                nc.sync.dma_start(out_p[0:64, lc * 512:(lc + 1) * 512], ofin[:])

            for lc in range(NLC):
                o65[lc] = oop.tile([65, 512], F32, tag='oo', name=f'o65_{lc}')
                nlc = lc + 1
                # stats micro-step schedule for chunk nlc: per lt:
                #   [mm h0][red h0][mm h1][red h1(offload 1 of 4)]
                steps = []
                if nlc < NLC:
                    sgs = {}
                    for i in range(4):
                        lt = 4 * nlc + i
                        steps.append(lambda lt=lt: sgs.__setitem__(lt, stats_mm(lt, 0)))
                        steps.append(lambda lt=lt: stats_red(lt, 0, sgs[lt]))
                        steps.append(lambda lt=lt: sgs.__setitem__(lt, stats_mm(lt, 1)))
                        steps.append(lambda lt=lt: stats_red(lt, 1, sgs[lt]))
                si = 0
                if steps:
                    steps[0]()
                    si = 1
                for g in range(NG):
                    emit_et_group(lc, g)
                    for _ in range(2):
                        if si < len(steps):
                            steps[si]()
                            si += 1
                    if g > 0:
                        emit_av(lc, g - 1)
                while si < len(steps):
                    steps[si]()
                    si += 1
                emit_av(lc, NG - 1)
                if nlc < NLC:
                    emit_brow(nlc)
                emit_norm(lc)

            if debug:
                nc.sync.dma_start(dbg_q_p[:], Q65[:].bitcast(F32))
                nc.sync.dma_start(dbg_k_p[:], K65[:].bitcast(F32))
                nc.sync.dma_start(dbg_sh_p[:], statsH[:])
                nc.sync.dma_start(dbg_ns_p[:], negst[:])

    nc.finalize()
    return nc


_cache = {}


def _get_nc(gamma: float):
    key = float(gamma)
    if key not in _cache:
        _cache[key] = build_nc(key)
    return _cache[key]


def _in_maps(inputs):
    x = np.asarray(inputs['x'], np.float32)
    vwT = f32r_round(np.asarray(inputs['value_w'], np.float32).T)
    qw = np.asarray(inputs['query_w'], np.float32)
    kw = np.asarray(inputs['key_w'], np.float32)
    mq = f32r_round((qw.T / SCALE) @ kw)
    ident = np.eye(128, dtype=np.float32)
    xs = f32r_round(x[..., 0])
    return [
        {'x': np.ascontiguousarray(xs[b]), 'vwT': vwT, 'mq': mq, 'ident': ident}
        for b in range(B)
    ]


def kernel(x, value_w, value_b, query_w, query_b, key_w, key_b, gamma):
    gamma_f = float(np.asarray(gamma).reshape(-1)[0])
    nc = _get_nc(gamma_f)
    maps = _in_maps(dict(x=x, value_w=value_w, query_w=query_w, key_w=key_w))
    res = run_bass_kernel_spmd(nc, maps, core_ids=list(range(B)), trace=False)
    out = np.stack([res.results[b]['out'] for b in range(B)], axis=0)
    return out[..., None].astype(np.float32)


def run_traced(inputs):
    gamma_f = float(np.asarray(inputs['gamma']).reshape(-1)[0])
    nc = _get_nc(gamma_f)
    maps = _in_maps(inputs)
    res = run_bass_kernel_spmd(nc, maps, core_ids=list(range(B)), trace=True)
    out = np.stack([res.results[b]['out'] for b in range(B)], axis=0)
    return out[..., None].astype(np.float32), res.exec_time_ns
